# revision 8
# baseline (speedup 1.0000x reference)
"""Trainium2 Bass kernel for nn_CCL_Module (3x3 cost-volume softmax flow).

Reference computation (per batch):
  c1 = l2norm_C(feature1); wp = l2norm_C(feature2) zero-padded spatially.
  match_vol[d=(dh,dw)] = sum_C c1 * shift(wp, dh, dw)      (9 shifts, 3x3)
  p = softmax(10 * match_vol, over d)
  flow_w = sum_d p * dw ; flow_h = sum_d p * dh
  out = concat([flow_w, flow_h])  -> [B, 2, H, W]

Strategy (pure data parallel, one batch per NeuronCore, 8 cores):
  - SBUF layout: H=128 on partitions, free dims = (C=64, W).
  - dh shifts  -> three h-shifted copies of feature2 loaded by DMA.
  - dw shifts  -> free-dim AP offsets into w-padded tiles.
  - Raw (unnormalized) dots A_d = sum_C f1 * shift(f2) via DVE
    tensor_mul + strided tensor_reduce (reduce innermost = C).
  - L2 normalization folded into score scaling:
      score_d = 10 * A_d * rsqrt(|f1|^2) * rsqrt(|f2|^2 shifted)
  - Scores are bounded by |10| so softmax needs no max subtraction:
      flow = (sum_d w_d * exp(s_d)) / (sum_d exp(s_d))
"""

import numpy as np

B, C, H, W = 8, 64, 128, 128
N_CORES = 8
SOFTMAX_SCALE = 10.0

_CACHE = {}


def _build_program(repeat: int = 1):
    import concourse.bass as bass
    import concourse.bacc as bacc
    import concourse.mybir as mybir
    from concourse.tile import TileContext
    from concourse.bass_utils import axon_active

    f32 = mybir.dt.float32
    nc = bacc.Bacc(
        "TRN2",
        target_bir_lowering=False,
        debug=not axon_active(),
        num_devices=N_CORES,
    )

    f1d = nc.declare_dram_parameter("feature1", [C, H, W], f32, isOutput=False)
    f2d = nc.declare_dram_parameter("feature2", [C, H, W], f32, isOutput=False)
    outd = nc.declare_dram_parameter("flow", [2, H, W], f32, isOutput=True)

    # DRAM views with h on the outer (partition) axis.
    f1v = f1d.rearrange("c h w -> h c w")
    f2v = f2d.rearrange("c h w -> h c w")
    outv = outd.rearrange("c h w -> h c w")

    # all-zero row used to zero-fill the dh edge partitions at load time
    zrow = nc.inline_tensor(np.zeros((1, C, W + 2), dtype=np.float32), name="zrow")

    with TileContext(nc) as tc:
        with tc.tile_pool(name="main", bufs=1) as pool:
          for _rep in range(repeat):
            # ---- input tiles ----
            xf1 = pool.tile([H, C, W], f32)          # f1, no padding
            # f2 with w padding (cols 0 and W+1), one tile per dh in {-1,0,1}.
            xf2_m = pool.tile([H, C, W + 2], f32)
            xf2_0 = pool.tile([H, C, W + 2], f32)
            xf2_p = pool.tile([H, C, W + 2], f32)

            nc.sync.dma_start(out=xf1[:, :, :], in_=f1v)
            # dh=0
            nc.sync.dma_start(out=xf2_0[:, :, 1 : W + 1], in_=f2v)
            # dh=-1: partition p holds f2 row p-1; row 0 is out of bounds -> 0
            nc.sync.dma_start(out=xf2_m[1:H, :, 1 : W + 1], in_=f2v[0 : H - 1])
            nc.sync.dma_start(out=xf2_m[0:1, :, :], in_=zrow[:])
            # dh=+1: partition p holds f2 row p+1; row H-1 out of bounds -> 0
            nc.sync.dma_start(out=xf2_p[0 : H - 1, :, 1 : W + 1], in_=f2v[1:H])
            nc.sync.dma_start(out=xf2_p[H - 1 : H, :, :], in_=zrow[:])

            # zero the w-pad columns so dw edge dots are exactly 0
            # (edge partitions already fully zeroed above; partition-0-based
            # memsets are legal for compute engines)
            for t in (xf2_m, xf2_0, xf2_p):
                nc.vector.memset(t[:, :, 0:1], 0.0)
                nc.vector.memset(t[:, :, W + 1 : W + 2], 0.0)

            xf2 = [xf2_m, xf2_0, xf2_p]

            # ---- raw correlation dots ----
            prod = pool.tile([H, C, W], f32)
            scoresA = pool.tile([H, 9, W], f32)     # A_d, d = dh*3+dw

            for d in range(9):
                dh, dw = d // 3 - 1, d % 3 - 1
                src = xf2[dh + 1][:, :, 1 + dw : 1 + dw + W]
                nc.vector.tensor_mul(prod[:, :, :], xf1[:, :, :], src)
                # reduce over C (innermost after permute)
                nc.vector.tensor_reduce(
                    scoresA[:, d, :],
                    prod.rearrange("h c w -> h w c"),
                    axis=mybir.AxisListType.X,
                    op=mybir.AluOpType.add,
                )

            # ---- norms ----
            r1sq = pool.tile([H, W], f32)
            r2m = pool.tile([H, W + 2], f32)  # |f2|^2 map, w-padded
            nc.vector.tensor_mul(prod[:, :, :], xf1[:, :, :], xf1[:, :, :])
            nc.vector.tensor_reduce(
                r1sq[:, :],
                prod.rearrange("h c w -> h w c"),
                axis=mybir.AxisListType.X,
                op=mybir.AluOpType.add,
            )
            f20 = xf2_0[:, :, 1 : W + 1]
            nc.vector.tensor_mul(prod[:, :, :], f20, f20)
            nc.vector.memset(r2m[:, 0:1], 1.0)
            nc.vector.memset(r2m[:, W + 1 : W + 2], 1.0)
            nc.vector.tensor_reduce(
                r2m[:, 1 : W + 1],
                prod.rearrange("h c w -> h w c"),
                axis=mybir.AxisListType.X,
                op=mybir.AluOpType.add,
            )

            # recip1 = 1/sqrt(r1sq), recip2 = 1/sqrt(r2m)
            recip1 = pool.tile([H, W], f32)
            recip2 = pool.tile([H, W + 2], f32)
            nc.scalar.sqrt(recip1[:, :], r1sq[:, :])
            nc.vector.reciprocal(recip1[:, :], recip1[:, :])
            nc.scalar.sqrt(recip2[:, :], r2m[:, :])
            nc.vector.reciprocal(recip2[:, :], recip2[:, :])

            # dh-shifted copies of recip2. Compute engines cannot address
            # partition-shifted APs, so shift across partitions via
            # SBUF->SBUF DMA. Edge rows clamp (their A is exactly 0).
            rec2_m = pool.tile([H, W + 2], f32)
            rec2_p = pool.tile([H, W + 2], f32)
            nc.sync.dma_start(out=rec2_m[1:H, :], in_=recip2[0 : H - 1, :])
            nc.sync.dma_start(out=rec2_m[0:1, :], in_=recip2[0:1, :])
            nc.sync.dma_start(out=rec2_p[0 : H - 1, :], in_=recip2[1:H, :])
            nc.sync.dma_start(out=rec2_p[H - 1 : H, :], in_=recip2[H - 1 : H, :])
            rec2 = [rec2_m, recip2, rec2_p]

            # ---- scores -> exp ----
            rmul = pool.tile([H, 9, W], f32)
            for d in range(9):
                dh, dw = d // 3 - 1, d % 3 - 1
                nc.vector.tensor_mul(
                    rmul[:, d, :], recip1[:, :], rec2[dh + 1][:, 1 + dw : 1 + dw + W]
                )
            expo = pool.tile([H, 9, W], f32)
            nc.vector.tensor_mul(rmul[:, :, :], rmul[:, :, :], scoresA[:, :, :])
            nc.scalar.activation(
                expo[:, :, :],
                rmul[:, :, :],
                mybir.ActivationFunctionType.Exp,
                scale=SOFTMAX_SCALE,
            )

            # ---- softmax-weighted displacement sums ----
            esum = pool.tile([H, W], f32)
            fwp = pool.tile([H, W], f32)
            fwm = pool.tile([H, W], f32)
            fhp = pool.tile([H, W], f32)
            fhm = pool.tile([H, W], f32)
            ex4 = expo.rearrange("h (a b) w -> h a b w", a=3)
            red = dict(axis=mybir.AxisListType.X, op=mybir.AluOpType.add)
            nc.vector.tensor_reduce(
                esum[:, :], expo.rearrange("h d w -> h w d"), **red
            )
            nc.vector.tensor_reduce(
                fwp[:, :], ex4[:, :, 2, :].rearrange("h a w -> h w a"), **red
            )
            nc.vector.tensor_reduce(
                fwm[:, :], ex4[:, :, 0, :].rearrange("h a w -> h w a"), **red
            )
            nc.vector.tensor_reduce(
                fhp[:, :], ex4[:, 2, :, :].rearrange("h b w -> h w b"), **red
            )
            nc.vector.tensor_reduce(
                fhm[:, :], ex4[:, 0, :, :].rearrange("h b w -> h w b"), **red
            )

            flows = pool.tile([H, 2, W], f32)
            nc.vector.reciprocal(esum[:, :], esum[:, :])
            nc.vector.tensor_sub(fwp[:, :], fwp[:, :], fwm[:, :])
            nc.vector.tensor_sub(fhp[:, :], fhp[:, :], fhm[:, :])
            nc.vector.tensor_mul(flows[:, 0, :], fwp[:, :], esum[:, :])
            nc.vector.tensor_mul(flows[:, 1, :], fhp[:, :], esum[:, :])

            nc.sync.dma_start(out=outv, in_=flows[:, :, :])

    nc.compile()
    return nc


def kernel(feature1: np.ndarray, feature2: np.ndarray) -> np.ndarray:
    from concourse import bass_utils

    if "nc" not in _CACHE:
        _CACHE["nc"] = _build_program()
    nc = _CACHE["nc"]

    f1 = np.ascontiguousarray(np.asarray(feature1, dtype=np.float32))
    f2 = np.ascontiguousarray(np.asarray(feature2, dtype=np.float32))
    in_maps = [
        {"feature1": f1[b], "feature2": f2[b]} for b in range(N_CORES)
    ]
    res = bass_utils.run_bass_kernel_spmd(nc, in_maps, list(range(N_CORES)))
    out = np.stack([res.results[b]["flow"] for b in range(N_CORES)], axis=0)
    return out.astype(np.float32)
